# revision 24
# baseline (speedup 1.0000x reference)
"""GCNN-alpha (gnn_message_passing) Trainium2 kernel.

Full-input contract: kernel(**inputs) takes the unsharded numpy inputs and
returns the [B, F_OUT] output. Internally:
  - Shards the graph-batch axis B=64 across 8 NeuronCores (8 graphs/core).
  - Host does integer-only edge preprocessing: per destination-row source
    lists with multiplicities (rows of S^T), packed/padded per partition.
  - On device, gpsimd.local_scatter builds S^T = (alpha*A + (1-alpha)*I)^T
    tiles directly in SBUF.
  - The two conv layers run as dense fp16 matmuls with fp32 PSUM
    accumulation, Horner's scheme out = P0 + S@(P1 + S@P2), P_k = Y@W_k,
    with a transpose-free structure that alternates matmul orientation:
      layer input Yt   [f=128 part, n=512 free]   (transposed)
      P0t  = lhsT W0, rhs Yt                      (1 mm, N=512, PSUM)
      P1u  = lhsT Yt-blk, rhs W1      per m-blk   (4 mm, N=128, PSUM)
      P2u  = lhsT Yt-blk, rhs W2      per m-blk   (4 mm, N=128, PSUM)
      u2   = copy P2u -> SBUF
      P1u += S@u2 via lhsT=S^T-blk, rhs=u2-blk    (16 mm, N=128)
      u1   = copy P1u -> SBUF
      P0t += (S@u1)^T via lhsT=u1-blk, rhs=S^T    (4 mm, N=512)
      Zt   = ACT ReLU(P0t + b)  (bias per-partition, fp16 out)
  - Max-pool is a free-axis reduce; readout is one matmul + K=1 bias mm.
"""

import os

import numpy as np
import ml_dtypes

import bass_rust
import concourse.bacc as bacc
import concourse.bass as bass
import concourse.tile as tile
from concourse import library_config, mybir
from concourse.bass_utils import run_bass_kernel_spmd

_add_dep = bass_rust.add_dep_helper

# Problem dims (hardcoded per spec).
B, n = 64, 512
F_IN, F_HID, F_OUT = 128, 128, 64
M = 8            # NeuronCores
GPC = B // M     # graphs per core
PB = n // 128    # 128-row partition blocks per graph
SPG = 2          # local_scatter calls per graph (num_elems=1024 each)
BPS = PB // SPG  # partition blocks covered per scatter call

_DT_NAME = os.environ.get("GNN_DT", "float16")
_REPEAT = int(os.environ.get("GNN_REPEAT", "1"))

_NP_DT = {
    "float16": np.float16,
    "bfloat16": ml_dtypes.bfloat16,
}[_DT_NAME]
_MY_DT = {
    "float16": mybir.dt.float16,
    "bfloat16": mybir.dt.bfloat16,
}[_DT_NAME]

_BUILD_CACHE = {}


def _build(k2: int):
    """Emit the per-core Bass program (identical on all 8 cores).
    k2 = num_idxs per local_scatter call (covers BPS row-blocks)."""
    DT = _MY_DT
    f32 = mybir.dt.float32
    nc = bacc.Bacc("TRN2", target_bir_lowering=False, debug=False,
                   num_devices=M)

    xt_d = nc.dram_tensor("xt", [128, GPC * n], DT, kind="ExternalInput")
    sidx_d = nc.dram_tensor("sidx", [128, GPC * SPG * k2], mybir.dt.int16,
                            kind="ExternalInput")
    sval_d = nc.dram_tensor("sval", [128, GPC * SPG * k2], DT,
                            kind="ExternalInput")
    w1_d = nc.dram_tensor("w1", [128, 3 * F_HID], DT, kind="ExternalInput")
    w2_d = nc.dram_tensor("w2", [128, 3 * F_HID], DT, kind="ExternalInput")
    wout_d = nc.dram_tensor("wout", [128, F_OUT], DT, kind="ExternalInput")
    b1_d = nc.dram_tensor("b1", [128, 1], f32, kind="ExternalInput")
    b2_d = nc.dram_tensor("b2", [128, 1], f32, kind="ExternalInput")
    bout_d = nc.dram_tensor("bout", [1, F_OUT], DT, kind="ExternalInput")
    out_d = nc.dram_tensor("out", [GPC, F_OUT], f32, kind="ExternalOutput")

    with tile.TileContext(nc) as tc:
        with (
            tc.tile_pool(name="consts", bufs=1) as consts,
            tc.tile_pool(name="st", bufs=3) as st_pool,
            tc.tile_pool(name="act", bufs=3) as act_pool,
            tc.tile_pool(name="u", bufs=6) as u_pool,
            tc.tile_pool(name="pp", bufs=2, space="PSUM") as pp,
        ):
            w1_t = consts.tile([128, 3 * F_HID], DT)
            nc.sync.dma_start(w1_t[:], w1_d.ap())
            w2_t = consts.tile([128, 3 * F_HID], DT)
            nc.sync.dma_start(w2_t[:], w2_d.ap())
            wout_t = consts.tile([128, F_OUT], DT)
            nc.sync.dma_start(wout_t[:], wout_d.ap())
            b1_t = consts.tile([128, 1], f32)
            nc.sync.dma_start(b1_t[:], b1_d.ap())
            b2_t = consts.tile([128, 1], f32)
            nc.sync.dma_start(b2_t[:], b2_d.ap())
            bout_t = consts.tile([1, F_OUT], DT)
            nc.sync.dma_start(bout_t[:], bout_d.ap())
            ones_t = consts.tile([1, GPC], DT)
            nc.vector.memset(ones_t[:], 1.0)

            sidx_t = consts.tile([128, GPC * SPG * k2], mybir.dt.int16)
            nc.sync.dma_start(sidx_t[:], sidx_d.ap())
            sval_t = consts.tile([128, GPC * SPG * k2], DT)
            nc.sync.dma_start(sval_t[:], sval_d.ap())
            xt_t = consts.tile([128, GPC * n], DT)
            nc.sync.dma_start(xt_t[:], xt_d.ap())

            nc.gpsimd.load_library(library_config.local_scatter)

            for rep in range(_REPEAT):
                pooledT = consts.tile([128, GPC], DT, tag="pooled")
                for g in range(GPC):
                    # Build S^T for this graph: SPG scatters of [128, BPS*n].
                    st_g = st_pool.tile([128, PB * n], DT, tag="st")
                    for s in range(SPG):
                        off = (g * SPG + s) * k2
                        nc.gpsimd.local_scatter(
                            out_ap=st_g[:, s * BPS * n:(s + 1) * BPS * n],
                            data_ap=sval_t[:, off:off + k2],
                            idxs_ap=sidx_t[:, off:off + k2],
                            channels=128,
                            num_elems=BPS * n,
                            num_idxs=k2,
                        )

                    yt = xt_t[:, g * n:(g + 1) * n]
                    for layer in range(2):
                        w_t = w1_t if layer == 0 else w2_t
                        b_t = b1_t if layer == 0 else b2_t
                        p0 = pp.tile([128, n], f32, tag="P0")   # transposed
                        # p12[:, mb*256:(mb+1)*256] = [P1u_mb | P2u_mb]
                        p12 = pp.tile([128, 2 * n], f32, tag="P12")
                        # P0t = W0^T @ Yt (transposed out, one N=512 mm)
                        nc.tensor.matmul(p0[:], w_t[:, 0:128], yt,
                                         start=True, stop=False)
                        # Merged W-products: [P1u_mb | P2u_mb] in one N=256
                        # mm per m-block (regions only read afterwards, so
                        # per-region start=True groups are fine).
                        for mb in range(PB):
                            ytb = yt[:, mb * 128:(mb + 1) * 128]
                            nc.tensor.matmul(
                                p12[:, mb * 256:(mb + 1) * 256], ytb,
                                w_t[:, 128:384],
                                start=True, stop=True,
                                skip_group_check=True)
                        # u2 = P2u (strided slices), on ACT to offload DVE
                        u2 = u_pool.tile([128, n], DT, tag="u")
                        p12_view = p12[:].rearrange(
                            "p (mb two f) -> p mb two f", two=2, f=128)
                        nc.scalar.copy(
                            u2[:].rearrange("p (mb f) -> p mb f", f=128),
                            p12_view[:, :, 1, :])
                        # hop1: T_mb = (S @ u2)_mb into its own psum tile;
                        # per-region chains led by kb=0 with start=True.
                        # start=True clears has_written for the WHOLE bank,
                        # so chains must not interleave -> ordering edges.
                        pt = pp.tile([128, n], f32, tag="T")
                        prev_last = None
                        for mb in range(PB):
                            for kb in range(PB):
                                mm = nc.tensor.matmul(
                                    pt[:, mb * 128:(mb + 1) * 128],
                                    st_g[:, kb * n + mb * 128:
                                         kb * n + (mb + 1) * 128],
                                    u2[:, kb * 128:(kb + 1) * 128],
                                    start=(kb == 0),
                                    stop=(kb == PB - 1),
                                    skip_group_check=True)
                                if kb == 0 and prev_last is not None:
                                    _add_dep(mm.ins, prev_last.ins,
                                             sync=False,
                                             reason="psum bank chain order")
                            prev_last = mm
                        # u1 = P1u + T (stage P1u in SBUF first: an op may
                        # read only one PSUM operand)
                        utmp = u_pool.tile([128, n], DT, tag="u")
                        nc.vector.tensor_copy(
                            utmp[:].rearrange("p (mb f) -> p mb f", f=128),
                            p12_view[:, :, 0, :])
                        u1 = u_pool.tile([128, n], DT, tag="u")
                        nc.vector.tensor_add(u1[:], utmp[:], pt[:])
                        # hop2: P0t += (S @ u1)^T (transposed out, N=512)
                        for kb in range(PB):
                            nc.tensor.matmul(
                                p0[:], u1[:, kb * 128:(kb + 1) * 128],
                                st_g[:, kb * n:(kb + 1) * n],
                                start=False, stop=(kb == PB - 1),
                                skip_group_check=True)
                        # ReLU(out + b) straight off PSUM, cast to DT
                        zt = act_pool.tile([128, n], DT, tag="z")
                        nc.scalar.activation(
                            zt[:], p0[:], mybir.ActivationFunctionType.Relu,
                            bias=b_t[:])
                        yt = zt[:]

                    nc.vector.tensor_reduce(
                        pooledT[:, g:g + 1], yt, axis=mybir.AxisListType.X,
                        op=mybir.AluOpType.max)

                # Readout: out = pooled @ Wout + bout
                po = pp.tile([GPC, F_OUT], f32, tag="T")
                nc.tensor.matmul(po[:], pooledT[:], wout_t[:],
                                 start=True, stop=False)
                nc.tensor.matmul(po[:], ones_t[:], bout_t[:],
                                 start=False, stop=True,
                                 skip_group_check=True)
                out_sb = consts.tile([GPC, F_OUT], f32, tag="osb")
                nc.vector.tensor_copy(out_sb[:], po[:])
                nc.sync.dma_start(out_d.ap(), out_sb[:])

    nc.compile()
    return nc


def _prep_scatter(edge_index: np.ndarray, alpha: float):
    """Integer-only edge prep. Returns per-scatter-call packed index/value
    arrays [B*n partition-rows grouped, SPG, k2] with trailing -1 padding.

    Row r of S^T (= dst node) has entries {src: alpha*mult} plus
    (1-alpha) added at column r. Scatter call s for graph g covers row
    blocks [s*BPS, (s+1)*BPS); partition p handles rows
    {g*n + s*BPS*128 + j*128 + p for j in range(BPS)} with column offset
    j*n inside the [128, BPS*n] output tile.
    """
    src = edge_index[0].astype(np.int64)
    dst = edge_index[1].astype(np.int64)
    g = src // n
    src_l = src - g * n
    dst_l = dst - g * n
    ekey = (g * n + dst_l) * n + src_l
    ukey, ucnt = np.unique(ekey, return_counts=True)
    uval = alpha * ucnt.astype(np.float64)
    rows_all = np.arange(B * n, dtype=np.int64)
    dkey = rows_all * n + (rows_all % n)
    pos = np.searchsorted(ukey, dkey)
    pos_c = np.minimum(pos, len(ukey) - 1)
    hit = ukey[pos_c] == dkey
    uval[pos_c[hit]] += 1.0 - alpha
    allk = np.concatenate([ukey, dkey[~hit]])
    allv = np.concatenate([uval, np.full((~hit).sum(), 1.0 - alpha)])
    order = np.argsort(allk, kind="stable")
    allk = allk[order]
    allv = allv[order]

    row = allk // n          # global row id in [0, B*n)
    col = allk % n           # column within the row's graph
    # Map each entry to (graph, scatter-call s, partition p, block j).
    r_l = row % n
    gg = row // n
    s = r_l // (BPS * 128)
    j = (r_l // 128) % BPS
    p = r_l % 128
    # partition-slot key: for (gg, s, p): entries packed ordered by (j, col)
    slot = (gg * SPG + s) * 128 + p
    order2 = np.lexsort((col, j, slot))
    slot = slot[order2]
    scat_col = (j * n + col)[order2].astype(np.int16)
    vals = allv[order2]

    nslots = B * SPG * 128
    slot_start = np.searchsorted(slot, np.arange(nslots))
    rank = np.arange(len(slot)) - slot_start[slot]
    k_need = int(rank.max()) + 1
    k2 = max(64, (k_need + 15) // 16 * 16)

    idx_arr = np.full((nslots, k2), -1, np.int16)
    val_arr = np.zeros((nslots, k2), np.float64)
    idx_arr[slot, rank] = scat_col
    val_arr[slot, rank] = vals
    # [B*SPG, 128, k2] -> group: graph-major per core handled in _make_in_maps
    return idx_arr.reshape(B, SPG, 128, k2), val_arr.reshape(B, SPG, 128, k2), k2


def _make_in_maps(inputs, idx_arr, val_arr):
    # idx/val: [B, SPG, 128, k2] -> per-core partition-major
    # [128, GPC*SPG*k2] with free order (g, s, k).
    def core_scatter(arr, np_dt):
        a = np.ascontiguousarray(arr).reshape(M, GPC, SPG, 128, -1)
        return [np.ascontiguousarray(
            a[c].transpose(2, 0, 1, 3).reshape(128, -1)).astype(np_dt)
            for c in range(M)]

    sidx = core_scatter(idx_arr, np.int16)
    sval = core_scatter(val_arr, _NP_DT)

    X = np.asarray(inputs["X"], np.float32)
    w1_h = np.ascontiguousarray(
        np.asarray(inputs["W1"], np.float32).transpose(1, 0, 2)
        .reshape(128, 3 * F_HID)).astype(_NP_DT)
    w2_h = np.ascontiguousarray(
        np.asarray(inputs["W2"], np.float32).transpose(1, 0, 2)
        .reshape(128, 3 * F_HID)).astype(_NP_DT)
    wout_h = np.asarray(inputs["Wout"], np.float32).astype(_NP_DT)
    b1_h = np.asarray(inputs["b1"], np.float32).reshape(128, 1)
    b2_h = np.asarray(inputs["b2"], np.float32).reshape(128, 1)
    bout_h = np.asarray(inputs["bout"], np.float32).reshape(1, F_OUT).astype(_NP_DT)

    in_maps = []
    for c in range(M):
        xc = X[c * GPC * n:(c + 1) * GPC * n].reshape(GPC, n, F_IN)
        xt = np.ascontiguousarray(
            xc.transpose(2, 0, 1).reshape(128, GPC * n)).astype(_NP_DT)
        in_maps.append({
            "xt": xt,
            "sidx": sidx[c],
            "sval": sval[c],
            "w1": w1_h,
            "w2": w2_h,
            "wout": wout_h,
            "b1": b1_h,
            "b2": b2_h,
            "bout": bout_h,
        })
    return in_maps


def kernel(X, batch, edge_index, W1, b1, W2, b2, Wout, bout, alpha):
    alpha_f = float(np.asarray(alpha))
    idx_arr, val_arr, k2 = _prep_scatter(np.asarray(edge_index), alpha_f)
    in_maps = _make_in_maps(
        {"X": X, "W1": W1, "b1": b1, "W2": W2, "b2": b2,
         "Wout": Wout, "bout": bout}, idx_arr, val_arr)

    key = (k2, _DT_NAME, _REPEAT)
    if key not in _BUILD_CACHE:
        _BUILD_CACHE[key] = _build(k2)
    nc = _BUILD_CACHE[key]

    res = run_bass_kernel_spmd(nc, in_maps, core_ids=list(range(M)))
    return np.concatenate([res.results[c]["out"] for c in range(M)], axis=0)
